# revision 34
# baseline (speedup 1.0000x reference)
"""Multi-label masked-gather mean loss on 8 Trainium2 NeuronCores.

reference:
    logp = log_softmax(x, -1); per_sample = -sum_t(mask*logp[i, y[i,t]])/count_i
    loss = mean(per_sample)

Identity used (count_i > 0):
    per_sample_i = logsumexp(x_i) - sum_t w[i,t] * x[i, y[i,t]],  w = mask/count
    loss = (sum_i logsumexp(x_i) + sum_{i,t} wneg[i,t] * x[i,y[i,t]]) / B
with wneg = -w. Data-parallel over the batch: 4096 rows -> 512 rows/core.

The exp+row-sum over the core's [512, 50257] fp8 shard is split across
engines so the scalar engine stops being the lone bottleneck:
  * ACT path (cols [0, CA)): row-major fp8 tiles, native Exp with fused
    accum_out per 128-row block.
  * DVE+PE path (cols [CA, C)): TRANSPOSED fp8 tiles [128 cols, 512 rows].
    DVE computes Schraudolph's bit-trick exp: int16(x*184.665 + B) viewed
    as bf16 IS approximately exp(x) (B tuned so the mean multiplicative
    bias over the fractional-exponent distribution is ~0). PE then sums
    along partitions (=columns) via an accumulating ones-matmul into
    PSUM[1, 512] = per-row partial sums.  The host pre-tiles the
    transposed shard as xt[p, chunk*512+e] = x[e, CA+chunk*128+p] so slab
    DMAs are plain 2D slices with 8KB-contiguous partition lines.
Per-element exp error ~2-4% is iid across ~25k elements per row-half, so
row sumexp error is <<0.1%; loss tolerance is 2e-2.

Both streams ramp with small first tiles (the 16 DMA engines take ~6us to
all come up; big leading transfers would stall the compute engines until
~16us). Host sums ACT partials + DVE partials per row, takes log, adds
the gathered-label term, divides by B.
"""

import sys

sys.path.insert(0, "/opt/trn_rl_repo")

import math

import numpy as np

import concourse.bass as bass
import concourse.tile as tile
from concourse import bacc, mybir
from concourse import bass_utils

# Problem shape (hardcoded per contract)
B, C, T = 4096, 50257, 8
NCORES = 8
BL = B // NCORES  # 512 rows per core
P = 128
RB = BL // P      # 4 row blocks per core
GCOLS = BL * T // P      # 32: gathered elements per partition

# Column split: last N_CHUNK*128 columns go to the DVE+PE path, first CA to
# ACT. Measured: ACT 0.93 ns/col (over 4 row blocks), DVE convert 0.549
# ns/col; this split ends both streams together.
N_CHUNK = 242
CT = N_CHUNK * P         # 30976 transposed cols
CA = C - CT              # 19281 ACT cols

# ACT tiles per row block: (width, n_dma_pieces). Row block 0 ramps up with
# small tiles so ACT starts ~5us in; all DMA units stay ~<=0.85MB — wider
# merged tiles (tried) hog the DMA engines and starve the slab stream.
ACT_TILES = {
    0: [(1024, 1), (2048, 1), (4096, 1), (6056, 1), (6057, 1)],
    1: [(6427, 1), (6427, 1), (6427, 1)],
    2: [(6427, 1), (6427, 1), (6427, 1)],
    3: [(6427, 1), (6427, 1), (6427, 1)],
}
for rb in range(RB):
    assert sum(w for w, _ in ACT_TILES[rb]) == CA
MAXW = max(w for tl in ACT_TILES.values() for w, _ in tl)
NT_BY_RB = [len(ACT_TILES[rb]) for rb in range(RB)]
COL0_BY_RB = [sum(NT_BY_RB[:rb]) for rb in range(RB)]
ACC_COLS = sum(NT_BY_RB)       # per-(rowblock, coltile) sumexp partials
OUT_COLS = ACC_COLS + 1        # + gather-dot col

# DVE+PE path slabs (chunks of 128 transposed cols x 512 rows). First slabs
# small so the first convert starts ~7us in; last slab moderate so the tail
# chain (convert -> matmuls -> psum DMA) is short.
SLABS = [4, 4] + [16] * 14 + [10]
assert sum(SLABS) == N_CHUNK
N_SLAB = len(SLABS)
K_CONV = 4                     # chunks per DVE convert instruction
SLAB_F = max(SLABS) * BL       # slab tile free size (8192)

# Schraudolph constants for bf16 bit patterns:
#   bits = x * 128*log2(e) + 128*(127 - c),  c = 0.0564298 zeroes the mean
#   multiplicative bias of the linear-mantissa approximation for f~U[0,1).
SCH_A = 128.0 * math.log2(math.e)          # 184.6650
SCH_B = 128.0 * (127.0 - 0.0564298)        # 16248.777

_f32 = mybir.dt.float32
_bf16 = mybir.dt.bfloat16
_fp8 = mybir.dt.float8e4
_i16 = mybir.dt.int16
_i32 = mybir.dt.int32

_compiled = None


def _build():
    nc = bacc.Bacc(
        "TRN2",
        target_bir_lowering=False,
        debug=False,
        enable_asserts=False,
        num_devices=NCORES,
    )
    x_t = nc.dram_tensor("x", [BL, C], _fp8, kind="ExternalInput")
    xt_t = nc.dram_tensor("xt", [P, N_CHUNK * BL], _fp8, kind="ExternalInput")
    idx_t = nc.dram_tensor("idx", [P, GCOLS], _i32, kind="ExternalInput")
    wneg_t = nc.dram_tensor("wneg", [P, GCOLS], _f32, kind="ExternalInput")
    # cols 0..ACC_COLS-1: ACT-path sumexp partials; col ACC_COLS: gather dot
    out_t = nc.dram_tensor("out", [P, OUT_COLS], _f32, kind="ExternalOutput")
    # DVE+PE path per-row partial sumexp
    dve_t = nc.dram_tensor("dve", [1, BL], _f32, kind="ExternalOutput")

    x = x_t.ap()
    xt = xt_t.ap()
    idx = idx_t.ap()
    wneg = wneg_t.ap()
    out = out_t.ap()
    dve = dve_t.ap()

    with tile.TileContext(nc) as tc:
        with (
            # deep input lookahead (~6MB ACT + ~8MB slabs): rides out the
            # cross-core HBM contention waves that otherwise stall engines
            # (slow reps showed identical busy but one DMA engine +12us).
            # tconv is not DMA'd (DVE->PE only, PE lags by <1 slab): 3 bufs.
            tc.tile_pool(name="xin", bufs=8) as xin_pool,
            tc.tile_pool(name="tin", bufs=10) as tin_pool,
            tc.tile_pool(name="tconv", bufs=3) as tconv_pool,
            tc.tile_pool(name="scratch", bufs=1) as scratch_pool,
            tc.tile_pool(name="stats", bufs=1) as stats_pool,
            tc.tile_pool(name="gather", bufs=1) as gather_pool,
            tc.psum_pool(name="psum", bufs=1) as psum_pool,
        ):
            # all ACT partials end up here and go out in one DMA
            acc = stats_pool.tile([P, OUT_COLS], _f32)
            # zero bias for Exp, zeroed by the scalar engine itself (a
            # gpsimd memset wouldn't run until ~6us and stalls ACT's start)
            bias0 = stats_pool.tile([P, 1], _f32)
            nc.scalar.memzero(bias0[:])
            # ones weights for the PE partition-sum (DVE is idle early)
            ones_w = stats_pool.tile([P, 1], _bf16)
            nc.vector.memset(ones_w[:], 1.0)

            # exp output scratch: values are unused, only accum_out matters
            exp_scratch = scratch_pool.tile([P, MAXW], mybir.dt.float8e4)

            # PE accumulates per-row sums here across all chunks
            prow = psum_pool.tile([1, BL], _f32)

            def act_tile(rb, ti, c0, cw, pieces):
                rows = slice(rb * P, (rb + 1) * P)
                # (tried: ACT tiles on the scalar engine's own HWDGE ring to
                # decouple the streams — uniformly ~10us slower; the arming
                # does NOT hide behind engine-busy. Keep one sync ring.)
                xtile = xin_pool.tile([P, MAXW], _fp8, tag="xt")
                for pc in range(pieces):
                    p0 = pc * cw // pieces
                    p1 = (pc + 1) * cw // pieces
                    nc.sync.dma_start(
                        out=xtile[:, p0:p1], in_=x[rows, c0 + p0 : c0 + p1]
                    )
                col = COL0_BY_RB[rb] + ti
                nc.scalar.activation(
                    out=exp_scratch[:, :cw],
                    in_=xtile[:, :cw],
                    func=mybir.ActivationFunctionType.Exp,
                    bias=bias0[:, 0:1],
                    accum_out=acc[:, col : col + 1],
                )

            def dve_slab(s):
                chunk0 = sum(SLABS[:s])
                k = SLABS[s]
                f_dram = chunk0 * BL
                last_slab = s == N_SLAB - 1
                # finer convert granularity on the last slab shortens the
                # tail chain (last convert -> matmuls -> psum DMA)
                kconv = 2 if last_slab else K_CONV
                tin = tin_pool.tile([P, SLAB_F], _fp8, tag="tin")
                nc.sync.dma_start(
                    out=tin[:, : k * BL], in_=xt[:, f_dram : f_dram + k * BL]
                )
                tcv = tconv_pool.tile([P, SLAB_F], _i16, tag="tconv")
                for ci in range((k + kconv - 1) // kconv):
                    f0 = ci * kconv * BL
                    kc = min(kconv, k - ci * kconv)
                    nc.vector.tensor_scalar(
                        out=tcv[:, f0 : f0 + kc * BL],
                        in0=tin[:, f0 : f0 + kc * BL],
                        scalar1=SCH_A,
                        scalar2=SCH_B,
                        op0=mybir.AluOpType.mult,
                        op1=mybir.AluOpType.add,
                    )
                    ebits = tcv[:, f0 : f0 + kc * BL].bitcast(_bf16)
                    for j in range(kc):
                        chunk = chunk0 + ci * kconv + j
                        nc.tensor.matmul(
                            out=prow[0:1, :],
                            lhsT=ones_w[:],
                            rhs=ebits[:, j * BL : (j + 1) * BL],
                            start=(chunk == 0),
                            stop=(chunk == N_CHUNK - 1),
                        )

            # --- gather path DMAs (tiny; SWDGE on gpsimd, no deps) ---
            idx_tile = gather_pool.tile([P, GCOLS], _i32)
            nc.gpsimd.dma_start(out=idx_tile[:], in_=idx[:])
            w_tile = gather_pool.tile([P, GCOLS], _f32)
            nc.gpsimd.dma_start(out=w_tile[:], in_=wneg[:])
            g_tile = gather_pool.tile([P, GCOLS], _fp8)
            nc.gpsimd.indirect_dma_start(
                out=g_tile[:],
                out_offset=None,
                in_=x[:],
                in_offset=bass.IndirectOffsetOnAxis(ap=idx_tile[:], axis=1),
            )

            def gather_compute():
                # ~1.3us of DVE work; run it early (after slab 1's converts)
                # so the final out DMA doesn't wait on it at the tail.
                g32 = gather_pool.tile([P, GCOLS], _f32)
                nc.vector.tensor_copy(out=g32[:], in_=g_tile[:])
                gw = gather_pool.tile([P, GCOLS], _f32)
                nc.vector.tensor_tensor(
                    out=gw[:], in0=g32[:], in1=w_tile[:], op=mybir.AluOpType.mult
                )
                nc.vector.tensor_reduce(
                    out=acc[:, ACC_COLS : ACC_COLS + 1],
                    in_=gw[:],
                    axis=mybir.AxisListType.X,
                    op=mybir.AluOpType.add,
                )

            # Interleaved issuance: alternate small ACT tiles and small
            # slabs first (DMA engines stagger up over ~6us), then steady
            # round-robin so both streams stay fed.
            act_list = []
            for rb in range(RB):
                c0 = 0
                for ti, (w, pieces) in enumerate(ACT_TILES[rb]):
                    act_list.append((rb, ti, c0, w, pieces))
                    c0 += w
            ai, si = 0, 0
            order = []
            while ai < len(act_list) or si < N_SLAB:
                if ai < len(act_list):
                    order.append(("A", act_list[ai]))
                    ai += 1
                if si < N_SLAB:
                    order.append(("S", si))
                    si += 1
            done_gather = False
            for kind, v in order:
                if kind == "A":
                    act_tile(*v)
                else:
                    dve_slab(v)
                    if v >= 1 and not done_gather:
                        gather_compute()
                        done_gather = True

            # acc out first: it only needs the last ACT accum + gather col,
            # and must not sit behind the matmul-gated drow copy.
            nc.scalar.dma_start(out=out[:], in_=acc[:])

            # DVE per-row partials: PSUM -> SBUF on the scalar engine (idle
            # by the tail; DVE may still be on its last convert), then out on
            # the same engine's ring — no cross-engine semaphore hop.
            drow = stats_pool.tile([1, BL], _f32)
            nc.scalar.copy(out=drow[:], in_=prow[0:1, :])
            nc.scalar.dma_start(out=dve[:], in_=drow[:])

    nc.compile()
    return nc


def _get_compiled():
    global _compiled
    if _compiled is None:
        _compiled = _build()
    return _compiled


def _make_in_maps(x, y):
    import ml_dtypes

    fp8 = ml_dtypes.float8_e4m3
    x = np.asarray(x, dtype=np.float32)
    y = np.asarray(y)
    mask = y != -1
    cnt = mask.sum(axis=1)
    # rows with count 0 would be NaN in the reference; inputs never hit this
    w = np.where(mask, 1.0 / np.maximum(cnt, 1)[:, None], 0.0).astype(np.float32)
    wneg = -w
    safe = np.where(mask, y, 0).astype(np.int64)

    in_maps = []
    for m in range(NCORES):
        sl = slice(m * BL, (m + 1) * BL)
        xs = np.ascontiguousarray(x[sl].astype(fp8))
        # pre-tiled transpose: xt[p, chunk*BL + e] = x[e, CA + chunk*P + p]
        xts = np.ascontiguousarray(
            x[sl, CA:]
            .T.astype(fp8)
            .reshape(N_CHUNK, P, BL)
            .transpose(1, 0, 2)
            .reshape(P, N_CHUNK * BL)
        )
        flat = (
            np.arange(BL, dtype=np.int64)[:, None] * C + safe[sl]
        ).astype(np.int32)
        in_maps.append(
            {
                "x": xs,
                "xt": xts,
                "idx": np.ascontiguousarray(flat.reshape(P, GCOLS)),
                "wneg": np.ascontiguousarray(wneg[sl].reshape(P, GCOLS)),
            }
        )
    return in_maps


def kernel(**inputs) -> np.ndarray:
    x, y = inputs["x"], inputs["y"]
    nc = _get_compiled()
    in_maps = _make_in_maps(x, y)
    res = bass_utils.run_bass_kernel_spmd(
        nc, in_maps, core_ids=list(range(NCORES))
    )
    total = 0.0
    for r in res.results:
        out = np.asarray(r["out"], dtype=np.float64)  # [P, OUT_COLS]
        drow = np.asarray(r["dve"], dtype=np.float64).reshape(BL)  # [BL]
        # per-row sumexp: ACT partials (variable cols per row block) + DVE
        for rb in range(RB):
            c0 = COL0_BY_RB[rb]
            se = out[:, c0 : c0 + NT_BY_RB[rb]].sum(axis=1)
            se = se + drow[rb * P : (rb + 1) * P]
            total += np.log(se).sum()
        total += out[:, ACC_COLS].sum()
    return np.float32(total / B)


# revision 35
# speedup vs baseline: 1.0194x; 1.0194x over previous
"""Multi-label masked-gather mean loss on 8 Trainium2 NeuronCores.

reference:
    logp = log_softmax(x, -1); per_sample = -sum_t(mask*logp[i, y[i,t]])/count_i
    loss = mean(per_sample)

Identity used (count_i > 0):
    per_sample_i = logsumexp(x_i) - sum_t w[i,t] * x[i, y[i,t]],  w = mask/count
    loss = (sum_i logsumexp(x_i) + sum_{i,t} wneg[i,t] * x[i,y[i,t]]) / B
with wneg = -w. Data-parallel over the batch: 4096 rows -> 512 rows/core.

The exp+row-sum over the core's [512, 50257] fp8 shard is split across
engines so the scalar engine stops being the lone bottleneck:
  * ACT path (cols [0, CA)): row-major fp8 tiles, native Exp with fused
    accum_out per 128-row block.
  * DVE+PE path (cols [CA, C)): TRANSPOSED fp8 tiles [128 cols, 512 rows].
    DVE computes Schraudolph's bit-trick exp: int16(x*184.665 + B) viewed
    as bf16 IS approximately exp(x) (B tuned so the mean multiplicative
    bias over the fractional-exponent distribution is ~0). PE then sums
    along partitions (=columns) via an accumulating ones-matmul into
    PSUM[1, 512] = per-row partial sums.  The host pre-tiles the
    transposed shard as xt[p, chunk*512+e] = x[e, CA+chunk*128+p] so slab
    DMAs are plain 2D slices with 8KB-contiguous partition lines.
Per-element exp error ~2-4% is iid across ~25k elements per row-half, so
row sumexp error is <<0.1%; loss tolerance is 2e-2.

Both streams ramp with small first tiles (the 16 DMA engines take ~6us to
all come up; big leading transfers would stall the compute engines until
~16us). Host sums ACT partials + DVE partials per row, takes log, adds
the gathered-label term, divides by B.
"""

import sys

sys.path.insert(0, "/opt/trn_rl_repo")

import math

import numpy as np

import concourse.bass as bass
import concourse.tile as tile
from concourse import bacc, mybir
from concourse import bass_utils

# Problem shape (hardcoded per contract)
B, C, T = 4096, 50257, 8
NCORES = 8
BL = B // NCORES  # 512 rows per core
P = 128
RB = BL // P      # 4 row blocks per core
GCOLS = BL * T // P      # 32: gathered elements per partition

# Column split: last N_CHUNK*128 columns go to the DVE+PE path, first CA to
# ACT. Measured: ACT 0.93 ns/col (over 4 row blocks), DVE convert 0.549
# ns/col; this split ends both streams together.
N_CHUNK = 242
CT = N_CHUNK * P         # 30976 transposed cols
CA = C - CT              # 19281 ACT cols

# ACT tiles per row block: (width, n_dma_pieces). Row block 0 ramps up with
# small tiles so ACT starts ~5us in; all DMA units stay ~<=0.85MB — wider
# merged tiles (tried) hog the DMA engines and starve the slab stream.
ACT_TILES = {
    0: [(1024, 1), (2048, 1), (4096, 1), (6056, 1), (6057, 1)],
    1: [(6427, 1), (6427, 1), (6427, 1)],
    2: [(6427, 1), (6427, 1), (6427, 1)],
    3: [(6427, 1), (6427, 1), (6427, 1)],
}
for rb in range(RB):
    assert sum(w for w, _ in ACT_TILES[rb]) == CA
MAXW = max(w for tl in ACT_TILES.values() for w, _ in tl)
NT_BY_RB = [len(ACT_TILES[rb]) for rb in range(RB)]
COL0_BY_RB = [sum(NT_BY_RB[:rb]) for rb in range(RB)]
ACC_COLS = sum(NT_BY_RB)       # per-(rowblock, coltile) sumexp partials
OUT_COLS = ACC_COLS + 1        # + gather-dot col

# DVE+PE path slabs (chunks of 128 transposed cols x 512 rows). First slabs
# small so the first convert starts ~7us in; last slab moderate so the tail
# chain (convert -> matmuls -> psum DMA) is short.
SLABS = [4, 4] + [16] * 14 + [10]
assert sum(SLABS) == N_CHUNK
N_SLAB = len(SLABS)
K_CONV = 4                     # chunks per DVE convert instruction
SLAB_F = max(SLABS) * BL       # slab tile free size (8192)

# Schraudolph constants for bf16 bit patterns:
#   bits = x * 128*log2(e) + 128*(127 - c),  c = 0.0564298 zeroes the mean
#   multiplicative bias of the linear-mantissa approximation for f~U[0,1).
SCH_A = 128.0 * math.log2(math.e)          # 184.6650
SCH_B = 128.0 * (127.0 - 0.0564298)        # 16248.777

_f32 = mybir.dt.float32
_bf16 = mybir.dt.bfloat16
_fp8 = mybir.dt.float8e4
_i16 = mybir.dt.int16
_i32 = mybir.dt.int32

_compiled = None


def _build():
    nc = bacc.Bacc(
        "TRN2",
        target_bir_lowering=False,
        debug=False,
        enable_asserts=False,
        num_devices=NCORES,
    )
    x_t = nc.dram_tensor("x", [BL, C], _fp8, kind="ExternalInput")
    xt_t = nc.dram_tensor("xt", [P, N_CHUNK * BL], _fp8, kind="ExternalInput")
    idx_t = nc.dram_tensor("idx", [P, GCOLS], _i32, kind="ExternalInput")
    wneg_t = nc.dram_tensor("wneg", [P, GCOLS], _f32, kind="ExternalInput")
    # cols 0..ACC_COLS-1: ACT-path sumexp partials; col ACC_COLS: gather dot
    out_t = nc.dram_tensor("out", [P, OUT_COLS], _f32, kind="ExternalOutput")
    # DVE+PE path per-row partial sumexp
    dve_t = nc.dram_tensor("dve", [1, BL], _f32, kind="ExternalOutput")

    x = x_t.ap()
    xt = xt_t.ap()
    idx = idx_t.ap()
    wneg = wneg_t.ap()
    out = out_t.ap()
    dve = dve_t.ap()

    with tile.TileContext(nc) as tc:
        with (
            # deep input lookahead (~6MB ACT + ~8MB slabs): rides out the
            # cross-core HBM contention waves that otherwise stall engines
            # (slow reps showed identical busy but one DMA engine +12us).
            # tconv is not DMA'd (DVE->PE only, PE lags by <1 slab): 3 bufs.
            tc.tile_pool(name="xin", bufs=8) as xin_pool,
            tc.tile_pool(name="tin", bufs=10) as tin_pool,
            tc.tile_pool(name="tconv", bufs=3) as tconv_pool,
            tc.tile_pool(name="scratch", bufs=1) as scratch_pool,
            tc.tile_pool(name="stats", bufs=1) as stats_pool,
            tc.tile_pool(name="gather", bufs=1) as gather_pool,
            tc.psum_pool(name="psum", bufs=1) as psum_pool,
        ):
            # all ACT partials end up here and go out in one DMA
            acc = stats_pool.tile([P, OUT_COLS], _f32)
            # zero bias for Exp, zeroed by the scalar engine itself (a
            # gpsimd memset wouldn't run until ~6us and stalls ACT's start)
            bias0 = stats_pool.tile([P, 1], _f32)
            nc.scalar.memzero(bias0[:])
            # ones weights for the PE partition-sum (DVE is idle early)
            ones_w = stats_pool.tile([P, 1], _bf16)
            nc.vector.memset(ones_w[:], 1.0)

            # exp output scratch: values are unused, only accum_out matters
            exp_scratch = scratch_pool.tile([P, MAXW], mybir.dt.float8e4)

            # PE accumulates per-row sums here across all chunks
            prow = psum_pool.tile([1, BL], _f32)

            def act_tile(rb, ti, c0, cw, pieces):
                rows = slice(rb * P, (rb + 1) * P)
                # (tried: ACT tiles on the scalar engine's own HWDGE ring to
                # decouple the streams — uniformly ~10us slower; the arming
                # does NOT hide behind engine-busy. Keep one sync ring.)
                xtile = xin_pool.tile([P, MAXW], _fp8, tag="xt")
                for pc in range(pieces):
                    p0 = pc * cw // pieces
                    p1 = (pc + 1) * cw // pieces
                    nc.sync.dma_start(
                        out=xtile[:, p0:p1], in_=x[rows, c0 + p0 : c0 + p1]
                    )
                col = COL0_BY_RB[rb] + ti
                nc.scalar.activation(
                    out=exp_scratch[:, :cw],
                    in_=xtile[:, :cw],
                    func=mybir.ActivationFunctionType.Exp,
                    bias=bias0[:, 0:1],
                    accum_out=acc[:, col : col + 1],
                )

            def dve_slab(s):
                chunk0 = sum(SLABS[:s])
                k = SLABS[s]
                f_dram = chunk0 * BL
                last_slab = s == N_SLAB - 1
                # finer convert granularity on the last slab shortens the
                # tail chain (last convert -> matmuls -> psum DMA)
                kconv = 2 if last_slab else K_CONV
                tin = tin_pool.tile([P, SLAB_F], _fp8, tag="tin")
                nc.sync.dma_start(
                    out=tin[:, : k * BL], in_=xt[:, f_dram : f_dram + k * BL]
                )
                tcv = tconv_pool.tile([P, SLAB_F], _i16, tag="tconv")
                for ci in range((k + kconv - 1) // kconv):
                    f0 = ci * kconv * BL
                    kc = min(kconv, k - ci * kconv)
                    nc.vector.tensor_scalar(
                        out=tcv[:, f0 : f0 + kc * BL],
                        in0=tin[:, f0 : f0 + kc * BL],
                        scalar1=SCH_A,
                        scalar2=SCH_B,
                        op0=mybir.AluOpType.mult,
                        op1=mybir.AluOpType.add,
                    )
                    ebits = tcv[:, f0 : f0 + kc * BL].bitcast(_bf16)
                    for j in range(kc):
                        chunk = chunk0 + ci * kconv + j
                        nc.tensor.matmul(
                            out=prow[0:1, :],
                            lhsT=ones_w[:],
                            rhs=ebits[:, j * BL : (j + 1) * BL],
                            start=(chunk == 0),
                            stop=(chunk == N_CHUNK - 1),
                        )

            # --- gather path DMAs (tiny; SWDGE on gpsimd, no deps) ---
            idx_tile = gather_pool.tile([P, GCOLS], _i32)
            nc.gpsimd.dma_start(out=idx_tile[:], in_=idx[:])
            w_tile = gather_pool.tile([P, GCOLS], _f32)
            nc.gpsimd.dma_start(out=w_tile[:], in_=wneg[:])
            g_tile = gather_pool.tile([P, GCOLS], _fp8)
            nc.gpsimd.indirect_dma_start(
                out=g_tile[:],
                out_offset=None,
                in_=x[:],
                in_offset=bass.IndirectOffsetOnAxis(ap=idx_tile[:], axis=1),
            )

            def gather_compute():
                # ~1.3us of DVE work; run it early (after slab 1's converts)
                # so the final out DMA doesn't wait on it at the tail.
                g32 = gather_pool.tile([P, GCOLS], _f32)
                nc.vector.tensor_copy(out=g32[:], in_=g_tile[:])
                gw = gather_pool.tile([P, GCOLS], _f32)
                nc.vector.tensor_tensor(
                    out=gw[:], in0=g32[:], in1=w_tile[:], op=mybir.AluOpType.mult
                )
                nc.vector.tensor_reduce(
                    out=acc[:, ACC_COLS : ACC_COLS + 1],
                    in_=gw[:],
                    axis=mybir.AxisListType.X,
                    op=mybir.AluOpType.add,
                )

            # Interleaved issuance: alternate small ACT tiles and small
            # slabs first (DMA engines stagger up over ~6us), then steady
            # round-robin so both streams stay fed.
            act_list = []
            for rb in range(RB):
                c0 = 0
                for ti, (w, pieces) in enumerate(ACT_TILES[rb]):
                    act_list.append((rb, ti, c0, w, pieces))
                    c0 += w
            # Slab 0 (small) leads: DVE's effective start is set by its
            # arrival (it trailed ACT's by ~4us when ACT tiles led), and
            # ACT's ramp tile only slips behind one 0.25MB transfer.
            ai, si = 0, 1
            order = [("S", 0)]
            while ai < len(act_list) or si < N_SLAB:
                if ai < len(act_list):
                    order.append(("A", act_list[ai]))
                    ai += 1
                if si < N_SLAB:
                    order.append(("S", si))
                    si += 1
            done_gather = False
            for kind, v in order:
                if kind == "A":
                    act_tile(*v)
                else:
                    dve_slab(v)
                    # gather ops go after slab 3's converts: DVE has no slack
                    # during the ramp, plenty once steady.
                    if v >= 3 and not done_gather:
                        gather_compute()
                        done_gather = True

            # acc out first: it only needs the last ACT accum + gather col,
            # and must not sit behind the matmul-gated drow copy.
            nc.scalar.dma_start(out=out[:], in_=acc[:])

            # DVE per-row partials: PSUM -> SBUF on the scalar engine (idle
            # by the tail; DVE may still be on its last convert), then out on
            # the same engine's ring — no cross-engine semaphore hop.
            drow = stats_pool.tile([1, BL], _f32)
            nc.scalar.copy(out=drow[:], in_=prow[0:1, :])
            nc.scalar.dma_start(out=dve[:], in_=drow[:])

    nc.compile()
    return nc


def _get_compiled():
    global _compiled
    if _compiled is None:
        _compiled = _build()
    return _compiled


def _make_in_maps(x, y):
    import ml_dtypes

    fp8 = ml_dtypes.float8_e4m3
    x = np.asarray(x, dtype=np.float32)
    y = np.asarray(y)
    mask = y != -1
    cnt = mask.sum(axis=1)
    # rows with count 0 would be NaN in the reference; inputs never hit this
    w = np.where(mask, 1.0 / np.maximum(cnt, 1)[:, None], 0.0).astype(np.float32)
    wneg = -w
    safe = np.where(mask, y, 0).astype(np.int64)

    in_maps = []
    for m in range(NCORES):
        sl = slice(m * BL, (m + 1) * BL)
        xs = np.ascontiguousarray(x[sl].astype(fp8))
        # pre-tiled transpose: xt[p, chunk*BL + e] = x[e, CA + chunk*P + p]
        xts = np.ascontiguousarray(
            x[sl, CA:]
            .T.astype(fp8)
            .reshape(N_CHUNK, P, BL)
            .transpose(1, 0, 2)
            .reshape(P, N_CHUNK * BL)
        )
        flat = (
            np.arange(BL, dtype=np.int64)[:, None] * C + safe[sl]
        ).astype(np.int32)
        in_maps.append(
            {
                "x": xs,
                "xt": xts,
                "idx": np.ascontiguousarray(flat.reshape(P, GCOLS)),
                "wneg": np.ascontiguousarray(wneg[sl].reshape(P, GCOLS)),
            }
        )
    return in_maps


def kernel(**inputs) -> np.ndarray:
    x, y = inputs["x"], inputs["y"]
    nc = _get_compiled()
    in_maps = _make_in_maps(x, y)
    res = bass_utils.run_bass_kernel_spmd(
        nc, in_maps, core_ids=list(range(NCORES))
    )
    total = 0.0
    for r in res.results:
        out = np.asarray(r["out"], dtype=np.float64)  # [P, OUT_COLS]
        drow = np.asarray(r["dve"], dtype=np.float64).reshape(BL)  # [BL]
        # per-row sumexp: ACT partials (variable cols per row block) + DVE
        for rb in range(RB):
            c0 = COL0_BY_RB[rb]
            se = out[:, c0 : c0 + NT_BY_RB[rb]].sum(axis=1)
            se = se + drow[rb * P : (rb + 1) * P]
            total += np.log(se).sum()
        total += out[:, ACC_COLS].sum()
    return np.float32(total / B)
